# revision 11
# baseline (speedup 1.0000x reference)
"""Bergman matrix layer TRN2 kernel (per-core program, batch-sharded).

Per core: hidden [T,1024] -> out [T,1024].
  m = hidden @ W_mat + b_mat                 (TensorE f32r fast path; W streamed
                                              once per 4-tile quad = 2x total)
  M = m/(||m||_F + 1e-5)*4   per (t,h)       (ACT Square-accum, scale in-place)
  lr/rl unnormalized matvec chains on DVE    (exact, T steps, both dirs fused)
  emission: v = u/||u||  (Pool decode + DVE reduce + ACT sqrt) -> DRAM scratch
  out = gelu(concat(v_lr, v_rl) @ W_out + b_out)

DRAM scratch layouts are chosen for DMA descriptor efficiency:
  m:  per t-tile [16 h, 128 t, 256 x]  (lr scan reads 16KB-contiguous spans)
  u:  per group  [32 (d,h), 16 i, 128 t], rl half stored t-reversed
      (out-phase gathers read 512B-contiguous t-runs)
"""

from contextlib import ExitStack

import concourse.bass as bass
import concourse.tile as tile
from concourse import mybir
from concourse.masks import make_identity


def _register_cum_matvec():
    import numpy as np
    from concourse.dve_spec import Spec, Src0, Src1, C1, scan, AluOp, lower
    from concourse.dve_uop import DveOpSpec
    import concourse.dve_ops as dve_ops
    from concourse.dve_ops import DveOp
    for op in dve_ops.OPS:
        if op.name == "CUM_MATVEC_ANT":
            return op

    def _ref(in0, in1, s0, s1, imm2):
        p = in0.shape[0]
        a = np.asarray(in0, dtype=np.float32).reshape(p, -1)
        b = np.asarray(in1, dtype=np.float32).reshape(p, -1)
        if isinstance(s1, np.ndarray):
            s1 = s1.reshape(p, -1)
        return np.cumsum(a * b * s1, axis=1).astype(np.float32)

    spec = Spec(body=scan(AluOp.ADD, Src0 * Src1 * C1), reference=_ref)
    op = DveOp("CUM_MATVEC_ANT", spec, subdim=False, uops_sha={})
    dve_ops.OPS.append(op)
    dve_ops._SUB_OPCODE_FOR_NAME[op.name] = (
        dve_ops._CUSTOM_DVE_ROW_BASE + len(dve_ops.OPS) - 1)
    if hasattr(dve_ops, "CUSTOM_DVE_SPECS"):
        dve_ops.CUSTOM_DVE_SPECS[op.name] = op.spec
    assert max(dve_ops._SUB_OPCODE_FOR_NAME.values()) < 0x20
    for ver in ("v3", "v4"):
        uops = lower(spec, ver=ver)
        opc = dve_ops.get_dve_sub_opcode(op.name)
        op.uops_sha[ver] = DveOpSpec(
            name=op.name, opcode=opc, uops=uops, rd1_en=True).sha(ver)
    return op


CUM_MATVEC = _register_cum_matvec()

AF = mybir.ActivationFunctionType
ALU = mybir.AluOpType
F32 = mybir.dt.float32
F32R = mybir.dt.float32r


def _r(ap):
    return ap.bitcast(F32R)

HID = 1024
NH = 16
NCOLS = 4096
MAT_EPS = 1e-5
SQRT_MD = 4.0
RESCALE = 8192.0
RESCALE_EVERY = 256


def bcast_dim(ap, n, axis):
    """Insert a step-0 dim of size n at position `axis` of an AP."""
    dims = [list(d) for d in ap.ap]
    dims.insert(axis, [0, n])
    return bass.AP(tensor=ap.tensor, offset=ap.offset, ap=dims)


def rev_dim(ap, axis):
    """Reverse dim `axis` of an AP."""
    dims = [list(d) for d in ap.ap]
    step, cnt = dims[axis]
    off = ap.offset + step * (cnt - 1)
    dims[axis] = [-step, cnt]
    return bass.AP(tensor=ap.tensor, offset=off, ap=dims)


def tails_view(scr):
    """[32, CH, 256] scan-stream tile -> [32, CH, 16] segment-tail view
    (elements at free positions i*16+15)."""
    dims = [list(d) for d in scr.ap]
    s = dims[-1][0]
    return bass.AP(tensor=scr.tensor, offset=scr.offset + 15 * s,
                   ap=[dims[0], dims[1], [16 * s, 16]])


def build_kernel(ctx: ExitStack, tc: tile.TileContext, T: int, gelu=True,
                 use_crit=False, use_f32r=True):
    nc = tc.nc
    NT = T // 128
    NGRP = T // 128
    CH = 16          # scan steps per mscan chunk
    assert NT % 4 == 0

    def mr(ap):
        return _r(ap) if use_f32r else ap

    hidden = nc.dram_tensor("hidden", [T, HID], F32, kind="ExternalInput").ap()
    w_mat = nc.dram_tensor("w_mat", [HID, NCOLS], F32, kind="ExternalInput").ap()
    b_mat = nc.dram_tensor("b_mat", [1, NCOLS], F32, kind="ExternalInput").ap()
    w_out = nc.dram_tensor("w_out", [512, HID], F32, kind="ExternalInput").ap()
    b_out = nc.dram_tensor("b_out", [1, HID], F32, kind="ExternalInput").ap()
    out = nc.dram_tensor("out", [T, HID], F32, kind="ExternalOutput").ap()

    singles = ctx.enter_context(tc.tile_pool(name="singles", bufs=1))
    hraw_p = ctx.enter_context(tc.tile_pool(name="hraw", bufs=2))
    ht_p = ctx.enter_context(tc.tile_pool(name="ht", bufs=4))
    w_p = ctx.enter_context(tc.tile_pool(name="wstr", bufs=2))
    mnorm_p = ctx.enter_context(tc.tile_pool(name="mnorm", bufs=2))
    s_p = ctx.enter_context(tc.tile_pool(name="stile", bufs=5))
    mscan_p = ctx.enter_context(tc.tile_pool(name="mscan", bufs=3))
    u2_p = ctx.enter_context(tc.tile_pool(name="u2", bufs=2))
    prod_p = ctx.enter_context(tc.tile_pool(name="prod", bufs=2))
    x_p = ctx.enter_context(tc.tile_pool(name="xtile", bufs=2))
    osb_p = ctx.enter_context(tc.tile_pool(name="osb", bufs=2))
    d_p = ctx.enter_context(tc.tile_pool(name="dtile", bufs=2))
    sqem_p = ctx.enter_context(tc.tile_pool(name="sqem", bufs=1))
    scr_p = ctx.enter_context(tc.tile_pool(name="scr", bufs=3))
    mdram_p = ctx.enter_context(tc.tile_pool(name="mdram", bufs=NT, space="DRAM"))
    udram_p = ctx.enter_context(tc.tile_pool(name="udram", bufs=NGRP, space="DRAM"))
    ps_tr = ctx.enter_context(tc.tile_pool(name="ps_tr", bufs=2, space="PSUM"))
    ps_mm = ctx.enter_context(tc.tile_pool(name="ps_mm", bufs=4, space="PSUM"))
    ps_out = ctx.enter_context(tc.tile_pool(name="ps_out", bufs=2, space="PSUM"))

    ident = singles.tile([128, 128], F32)
    make_identity(nc, ident)
    ones_row = singles.tile([1, 128], F32)
    nc.vector.memset(ones_row, 1.0)
    bmat_sb = singles.tile([1, NCOLS], F32)
    nc.sync.dma_start(out=bmat_sb, in_=b_mat)
    bout_sb = singles.tile([1, HID], F32)
    nc.sync.dma_start(out=bout_sb, in_=b_out)
    wout_sb = singles.tile([128, 4, HID], F32R)
    for kt in range(4):
        nc.sync.dma_start(out=wout_sb[:, kt, :],
                          in_=w_out[kt * 128:(kt + 1) * 128, :].bitcast(F32R))
    w_init = singles.tile([32, 16], F32)
    nc.vector.memset(w_init, 1.0)

    m_tiles = {}     # tt -> DRAM tile [16, 128, 256] delta-encoded m (h, t, x)
    u_dram = {}      # grp -> DRAM tile [32, 16, 128] normalized v (dh, i, t)

    def phase1_quad(q):
        """Compute delta-encoded normalized m for t-tiles
        {2q, 2q+1, NT-2-2q, NT-1-2q}; one W stream pass per quad."""
        tts = sorted({2 * q, 2 * q + 1, NT - 2 - 2 * q, NT - 1 - 2 * q})
        hts = {}
        for tt in tts:
            hraw = hraw_p.tile([128, HID], F32, tag="hraw")
            nc.sync.dma_start(out=hraw, in_=hidden[tt * 128:(tt + 1) * 128, :])
            ht = ht_p.tile([128, 8, 128], F32, tag="ht", name=f"ht{tt}")
            for kc in range(8):
                ps = ps_tr.tile([128, 128], F32, tag="tr")
                nc.tensor.transpose(ps, hraw[:, kc * 128:(kc + 1) * 128], ident)
                nc.scalar.activation(ht[:, kc, :], ps, AF.Copy)
            hts[tt] = ht
        sts = {}
        for tt in tts:
            sts[tt] = s_p.tile([128, NH], F32, tag="stile", name=f"st{tt}")
            m_tiles[tt] = mdram_p.tile([16, 128, 256], F32, tag=f"md{tt}",
                                       name=f"md{tt}")
        for cg in range(8):
            col0 = cg * 512
            pss = {tt: ps_mm.tile([128, 512], F32, tag="mm",
                                  name=f"psmm{tt}_{cg}") for tt in tts}
            for kc in range(8):
                wsl = w_p.tile([128, 512], F32, tag="wstr")
                nc.sync.dma_start(
                    out=wsl, in_=w_mat[kc * 128:(kc + 1) * 128, col0:col0 + 512])
                for tt in tts:
                    nc.tensor.matmul(pss[tt], mr(hts[tt][:, kc, :]), mr(wsl),
                                     start=(kc == 0), stop=False)
            for tt in tts:
                nc.tensor.matmul(pss[tt], mr(ones_row),
                                 mr(bmat_sb[:, col0:col0 + 512]),
                                 start=False, stop=True)
            for tt in tts:
                ps, st = pss[tt], sts[tt]
                mn = mnorm_p.tile([128, 512], F32, tag="mnorm",
                                  name=f"mn{tt}_{cg}")
                ssl = st[:, cg * 2:cg * 2 + 2]
                for hh in range(2):
                    scr = prod_p.tile([128, 256], F32, tag="sq_scr")
                    nc.scalar.activation(scr, ps[:, hh * 256:(hh + 1) * 256],
                                         AF.Square,
                                         accum_out=st[:, cg * 2 + hh:cg * 2 + hh + 1])
                nc.scalar.activation(ssl, ssl, AF.Sqrt)
                nc.vector.tensor_scalar_add(ssl, ssl, MAT_EPS)
                nc.vector.reciprocal(ssl, ssl)
                nc.vector.tensor_scalar_mul(ssl, ssl, SQRT_MD)
                for hh in range(2):
                    nc.scalar.activation(mn[:, hh * 256:(hh + 1) * 256],
                                         ps[:, hh * 256:(hh + 1) * 256], AF.Copy,
                                         scale=st[:, cg * 2 + hh:cg * 2 + hh + 1])
                dt_ = d_p.tile([128, 512], F32, tag="dtile", name=f"d{tt}_{cg}")
                nc.gpsimd.tensor_tensor(dt_[:, 0:511], mn[:, 0:511],
                                        mn[:, 1:512], op=ALU.subtract)
                mn_j15 = mn.rearrange("p (a j) -> p a j", j=16)[:, :, 15]
                dt_j15 = dt_.rearrange("p (a j) -> p a j", j=16)[:, :, 15]
                nc.gpsimd.tensor_copy(dt_j15, mn_j15)
                # write [t, h2, x] -> DRAM [h2, t, x] (dst view iterates t-major)
                dst = m_tiles[tt][2 * cg:2 * cg + 2, :, :].transpose([1, 0, 2])
                nc.gpsimd.dma_start(
                    out=dst, in_=dt_.rearrange("p (h x) -> p h x", h=2))

    def mscan_chunk(c):
        """[32=(d,h), CH tau, 256]; d=0: t=CH*c+tau, d=1: t=T-1-CH*c-tau."""
        mt = mscan_p.tile([32, CH, 256], F32, tag="mscan", name=f"mt{c}")
        t0 = CH * c
        g = t0 // 128
        r0 = t0 - 128 * g
        # lr half: [16 h, CH t, 256 x], contiguous (t, x) spans per head
        nc.scalar.dma_start(out=mt[0:16, :, :],
                            in_=m_tiles[g][:, r0:r0 + CH, :])
        # rl half: t = thi - tau (descending) -> negative t stride
        thi = T - 1 - t0
        g2 = thi // 128
        r1 = thi - 128 * g2
        src = m_tiles[g2][:, r1 - CH + 1:r1 + 1, :]
        nc.gpsimd.dma_start(out=mt[16:32, :, :], in_=rev_dim(src, 1))
        return mt

    scan_state = {}

    def scan_steps(mt, scr, c):
        """Run CH scan steps for chunk c: mt -> scr (DVE only)."""
        for j in range(CH):
            tau = CH * c + j
            if tau == 0:
                wb = bcast_dim(w_init, 16, 1)
            else:
                pt = scan_state["prev_tail"]
                wb = bcast_dim(pt, 16, 1)
            m_in = mt[:, j, :].rearrange("p (i x) -> p i x", i=16)
            s1 = RESCALE if (tau % RESCALE_EVERY == 0 and tau > 0) else 1.0
            nc.vector._custom_dve(
                CUM_MATVEC, out=scr[:, j, :].rearrange("p (i x) -> p i x", i=16),
                in0=m_in, in1=wb, s1=s1)
            dims = [list(d) for d in scr.ap]
            s = dims[-1][0]
            scan_state["prev_tail"] = bass.AP(
                tensor=scr.tensor,
                offset=scr.offset + dims[1][0] * j + 15 * s,
                ap=[dims[0], [16 * s, 16]])

    def decode_chunk(scr, u2, cc):
        """u[i] = W[i] - W[i-1] from scr tails into u2 [32, 16 i, 128 t].
        rl half (partitions 16:32) stored t-reversed within the group."""
        wt = tails_view(scr).transpose([0, 2, 1])   # [32, 16 i, CH t]
        lr = u2[0:16, :, cc * CH:(cc + 1) * CH]
        nc.gpsimd.tensor_tensor(lr[:, 1:16, :], wt[0:16, 1:16, :],
                                wt[0:16, 0:15, :], op=ALU.subtract)
        nc.gpsimd.tensor_copy(lr[:, 0:1, :], wt[0:16, 0:1, :])
        rl = rev_dim(u2[16:32, :, (7 - cc) * CH:(8 - cc) * CH], 2)
        nc.gpsimd.tensor_tensor(rl[:, 1:16, :], wt[16:32, 1:16, :],
                                wt[16:32, 0:15, :], op=ALU.subtract)
        nc.gpsimd.tensor_copy(rl[:, 0:1, :], wt[16:32, 0:1, :])

    def emit_group(grp, u2):
        """normalize: u2 *= 1/||u|| (per (dh, t)), then DMA to DRAM scratch."""
        nrm = s_p.tile([32, 128], F32, tag="nrm")
        sq = sqem_p.tile([32, 16, 128], F32, tag="sq_em")
        nc.scalar.activation(sq, u2, AF.Square)
        nc.vector.reduce_sum(out=nrm, in_=sq.transpose([0, 2, 1]),
                             axis=mybir.AxisListType.X)
        nc.scalar.activation(nrm, nrm, AF.Sqrt)
        nc.vector.reciprocal(nrm, nrm)
        nc.gpsimd.tensor_tensor(u2, u2, bcast_dim(nrm, 16, 1), op=ALU.mult)
        ud = udram_p.tile([32, 16, 128], F32, tag=f"ud{grp}", name=f"ud{grp}")
        u_dram[grp] = ud
        nc.gpsimd.dma_start(out=ud, in_=u2)

    def out_block(b):
        """out rows [128b, 128b+128): lr from grp b, rl from grp NGRP-1-b
        (already t-reversed in its u scratch)."""
        glr = u_dram[b]
        grl = u_dram[NGRP - 1 - b]
        xk = x_p.tile([128, 4, 128], F32R, tag="xtile")   # [(h4 d i), kt, t]
        xv = xk.rearrange("(h4 d i) k t -> h4 d i k t", h4=4, d=2)
        for hh in range(4):
            # src [16 i, 4 kt(h=4kt+hh), 128 t]
            src_l = glr.rearrange("(d kt h4) i t -> d h4 i kt t", d=2, kt=4)
            nc.sync.dma_start(out=xv[hh, 0], in_=src_l[0, hh].bitcast(F32R))
            src_r = grl.rearrange("(d kt h4) i t -> d h4 i kt t", d=2, kt=4)
            nc.sync.dma_start(out=xv[hh, 1], in_=src_r[1, hh].bitcast(F32R))
        for oc in range(2):
            ps = ps_out.tile([128, 512], F32, tag="po")
            for kt in range(4):
                nc.tensor.matmul(ps, xk[:, kt, :],
                                 wout_sb[:, kt, oc * 512:(oc + 1) * 512],
                                 start=(kt == 0), stop=False)
            nc.tensor.matmul(ps, mr(ones_row),
                             mr(bout_sb[:, oc * 512:(oc + 1) * 512]),
                             start=False, stop=True)
            osb = osb_p.tile([128, 512], F32, tag="osb")
            nc.scalar.activation(osb, ps, AF.Gelu if gelu else AF.Identity)
            nc.scalar.dma_start(
                out=out[b * 128:(b + 1) * 128, oc * 512:(oc + 1) * 512],
                in_=osb)

    # ---- flat software-pipelined chunk loop -------------------------------
    NCHUNK = NGRP * 8
    done = set()
    phase1_quad(0)
    mts = {}
    mts[0] = mscan_chunk(0)
    mts[1] = mscan_chunk(1)
    u2_cur = {}
    for c in range(NCHUNK):
        grp = c // 8
        if c % 8 == 0:
            u2_cur[grp] = u2_p.tile([32, 16, 128], F32, tag="u2",
                                    name=f"u2_{grp}")
        if c + 2 < NCHUNK:
            mts[c + 2] = mscan_chunk(c + 2)
        if c % 16 == 0 and c // 16 + 1 < NT // 4:
            phase1_quad(c // 16 + 1)
        scr = scr_p.tile([32, CH, 256], F32, tag="scr", name=f"scr{c}")
        if use_crit:
            with tc.tile_critical(name=f"sg{c}"):
                scan_steps(mts[c], scr, c)
        else:
            scan_steps(mts[c], scr, c)
        del mts[c]
        decode_chunk(scr, u2_cur[grp], c % 8)
        if c % 8 == 7:
            emit_group(grp, u2_cur.pop(grp))
            mirror = NGRP - 1 - grp
            if mirror in u_dram and mirror not in done:
                done.add(grp); done.add(mirror)
                out_block(min(grp, mirror))
                if mirror != grp:
                    out_block(max(grp, mirror))


def build_nc(T=2048, gelu=True, use_crit=False, use_f32r=True):
    import concourse.bacc as bacc
    nc = bacc.Bacc("TRN2", target_bir_lowering=False, debug=False)
    with tile.TileContext(nc) as tc:
        with ExitStack() as ctx:
            build_kernel(ctx, tc, T, gelu=gelu, use_crit=use_crit,
                         use_f32r=use_f32r)
    nc.compile()
    return nc


# ----------------------------------------------------------------------------
# Self-contained entry point: full inputs in, full outputs out (8 cores).
# ----------------------------------------------------------------------------
import numpy as np

_NC_CACHE = {}


def _get_nc(T):
    if T not in _NC_CACHE:
        _NC_CACHE[T] = build_nc(T=T, gelu=True)
    return _NC_CACHE[T]


def kernel(hidden_states, W_mat, b_mat, W_out, b_out):
    from concourse.bass_utils import run_bass_kernel_spmd
    B, T, _ = hidden_states.shape
    nc = _get_nc(T)
    w_mat = np.ascontiguousarray(W_mat, dtype=np.float32)
    b_mat_ = np.ascontiguousarray(b_mat, dtype=np.float32).reshape(1, -1)
    w_out = np.ascontiguousarray(W_out, dtype=np.float32)
    b_out_ = np.ascontiguousarray(b_out, dtype=np.float32).reshape(1, -1)
    in_maps = [
        {
            "hidden": np.ascontiguousarray(hidden_states[b], dtype=np.float32),
            "w_mat": w_mat,
            "b_mat": b_mat_,
            "w_out": w_out,
            "b_out": b_out_,
        }
        for b in range(B)
    ]
    res = run_bass_kernel_spmd(nc, in_maps, list(range(B)))
    return np.stack([res.results[b]["out"] for b in range(B)], axis=0)
